# revision 20
# baseline (speedup 1.0000x reference)
"""Trainium2 Bass kernel: multi-head causal attention (B=2, T=2048, C=1024, H=16).

Sharding: 8 cores = data parallel over B (2) x tensor parallel over head
groups (4 groups of 4 heads).  Each core computes its batch's partial
output contribution from its 4 heads through Wo rows; the host sums the 4
partials per batch (the "all-reduce") and adds the folded biases.

Device pipeline (per core, 4 heads; matmul operands bf16, PSUM fp32):
  - Q/K/V arrive HOST-pre-transposed as [C, T] bf16, so projections need no
    on-chip transposes: qT/kT = Wq^T @ X^T laid out [head_dim, T] directly,
    v natural [T, dv] with an extra always-1.0 65th column per head.
  - scores are computed TRANSPOSED: scT[k, q] = kT_blk^T @ qT_chunk, one
    512-wide PE matmul per (key-block, query-chunk); diagonal strips are
    partial-width.
  - one Exp (scale=1/8) per PSUM pair-strip writes expT[k, q] bf16 (no
    normalization yet); the diagonal block's strict-lower (q < k) entries are
    then zeroed by a 0/1-triangle multiply on the idle GPSIMD engine.
  - weights/constants are DMA'd once outside the reps loop (SBUF-resident);
    heads are software-pipelined with attnv matmuls of head h-1 interleaved
    BETWEEN the score-pair strips of head h, so the in-order PE stream always
    has ready work while the Activation engine catches up on Exp.
  - attn@v: out[q, dv+1] accumulates expT_blk^T @ [v|1] over key blocks; the
    65th column is the softmax row-sum for free.  out = out[:, :64] * (1/sum)
    via one broadcast DVE multiply per (chunk, head).
  - per chunk: PE-transpose out -> outT[dims, q]; output projection
    fin[q, C] = outT^T @ Wo streams wide; DMA fin (bf16) to DRAM.
  - reps>1 (timing mode): the For_i body holds TWO kernel bodies with
    parity-alternated qT/kT/v65 tiles (breaks the cross-rep WAR hazard) and
    staggered_reset=True (no all-engine barrier at the back edge), so
    consecutive reps pipeline: rep i+1's input DMAs + projections overlap
    rep i's Act-bound attention tail.
"""

from contextlib import ExitStack

import numpy as np
import ml_dtypes

import concourse.bass as bass
import concourse.mybir as mybir
import concourse.tile as tile
from concourse import bacc
from concourse.bass_utils import run_bass_kernel_spmd

B, T, C = 2, 2048, 1024
H, DK, DV = 16, 64, 64
N_CORES = 8
GROUPS = 4                 # head groups (tensor parallel)
HPG = H // GROUPS          # 4 heads per group
GD = HPG * DK              # 256 head dims per group
P = 128
TCH = 512                  # query chunk for attention
NCB = C // P               # contraction chunks over C

BF = mybir.dt.bfloat16
F32 = mybir.dt.float32
AX = mybir.AxisListType
AF = mybir.ActivationFunctionType

bf16 = ml_dtypes.bfloat16

CFG = {"xin_bufs": 2, "sc_bufs": 2, "mm_bufs": 2, "out4_bufs": 2,
       "expt_bufs": 3, "fin_bufs": 3, "osb_bufs": 2,
       "parity": True, "staggered": True, "out_bf16": True,
       # "block": proj block at body start (cross-rep overlap via parity +
       # staggered stages); "desc": descending chunks with fine-grained
       # proj/wo unit interleave (better in the cost model, worse on HW)
       "schedule": "block",
       # fp8e4m3 q/k + DoubleRow scores matmuls (0.5 cycles/row on PE).
       # Measured SLOWER on HW (150.3us vs 128.8): the 256-wide DoubleRow
       # stationary loads exceed their 128-cycle matmuls (Ldweights-bound).
       "fp8_scores": False}

OUT_DT = BF if CFG["out_bf16"] else F32
F8 = mybir.dt.float8e4
QK_DT = F8 if CFG["fp8_scores"] else BF
DR = mybir.MatmulPerfMode.DoubleRow


def _emit_consts(nc, tc, io, t_len, ctx):
    """Loop-invariant setup: weights/constants DMA + persistent activation
    tiles (x2 for rep-parity double buffering) + the ones column."""
    NT = t_len // P
    cpool = ctx.enter_context(tc.tile_pool(name="const", bufs=1))
    ppool = ctx.enter_context(tc.tile_pool(name="pers", bufs=1))

    ident = cpool.tile([P, P], BF)
    nc.sync.dma_start(out=ident, in_=io["ident"][:, :])
    triT = cpool.tile([P, P], BF)   # keep-mask 0/1 on the causal diagonal
    nc.sync.dma_start(out=triT, in_=io["triT"][:, :])
    bq_sb = cpool.tile([P, 2], F32)
    nc.sync.dma_start(out=bq_sb, in_=io["bq"][:, :])
    bk_sb = cpool.tile([P, 2], F32)
    nc.sync.dma_start(out=bk_sb, in_=io["bk"][:, :])

    wq_sb = cpool.tile([P, NCB, GD], BF)
    wk_sb = cpool.tile([P, NCB, GD], BF)
    wv_sb = cpool.tile([P, NCB, GD], BF)
    for w_sb, name in ((wq_sb, "wq"), (wk_sb, "wk"), (wv_sb, "wv")):
        nc.sync.dma_start(
            out=w_sb,
            in_=io[name][:, :].rearrange("(c p) d -> p c d", p=P))
    wo_sb = cpool.tile([P, 2, C], BF)
    nc.sync.dma_start(
        out=wo_sb, in_=io["wo"][:, :].rearrange("(r p) d -> p r d", p=P))

    npar = 2 if CFG["parity"] else 1
    # fp8 mode: partition rho = h*32+pp, free (i, t) holds head-dim i*32+pp
    # (host permutes Wq/Wk columns to match) — the DoubleRow scores layout
    qT_sb = [ppool.tile([P, 2, t_len], QK_DT, name=f"qT_sb{i}")
             for i in range(npar)]
    kT_sb = [ppool.tile([P, 2, t_len], QK_DT, name=f"kT_sb{i}")
             for i in range(npar)]
    v65_sb = [ppool.tile([P, NT, HPG, DV + 1], BF, name=f"v65_sb{i}")
              for i in range(npar)]
    outT_sb = ppool.tile([P, 2, t_len], BF)  # [pair head dims, pair, T]
    for v65 in v65_sb:
        nc.gpsimd.memset(v65[:, :, :, DV:DV + 1], 1.0)

    # shared streaming pools (created ONCE; both parity bodies rotate the
    # same tag ring buffers, which also gives cross-body WAR tracking)
    spool = ctx.enter_context(tc.tile_pool(name="stream", bufs=2))
    apool = ctx.enter_context(tc.tile_pool(name="attn", bufs=2))
    pp = ctx.enter_context(tc.tile_pool(name="ps", bufs=2, space="PSUM"))

    return {"ident": ident, "triT": triT, "bq": bq_sb, "bk": bk_sb,
            "wq": wq_sb, "wk": wk_sb, "wv": wv_sb, "wo": wo_sb,
            "qT": qT_sb, "kT": kT_sb, "v65": v65_sb, "outT": outT_sb,
            "spool": spool, "apool": apool, "pp": pp}


def _proj_units(nc, io, c, par, t4, with_dma):
    """One query-chunk's worth of projection work as a list of closures.

    Each unit is an atomic emission (one PSUM accumulation group + its
    drain); the caller interleaves units into the attention PE stream so the
    in-order PE queue always has ready work while Activation catches up on
    Exp and DVE drains the previous PSUM group.
    """
    spool, pp = c["spool"], c["pp"]
    bq_sb, bk_sb = c["bq"], c["bk"]
    wq_sb, wk_sb, wv_sb = c["wq"], c["wk"], c["wv"]
    qT_sb, kT_sb, v65_sb = c["qT"][par], c["kT"][par], c["v65"][par]
    t0 = t4 * TCH
    xq = spool.tile([P, NCB, TCH], BF, tag="xq", bufs=CFG["xin_bufs"],
                    name=f"xq{par}_{t4}")
    xk = spool.tile([P, NCB, TCH], BF, tag="xk", bufs=CFG["xin_bufs"],
                    name=f"xk{par}_{t4}")
    xv = spool.tile([P, NCB, TCH], BF, tag="xv", bufs=CFG["xin_bufs"],
                    name=f"xv{par}_{t4}")
    units = []

    def dma_unit():
        for x_sb, name in ((xq, "qT"), (xk, "kT"), (xv, "vT")):
            nc.sync.dma_start(
                out=x_sb,
                in_=io[name][:, t0:t0 + TCH].rearrange(
                    "(c p) t -> p c t", p=P))
    if with_dma:
        units.append(dma_unit)

    def qk_unit(x_sb, w_sb, bias_sb, xT_sb, pr):
        def emit():
            ps = pp.tile([P, TCH], F32, tag="mm", bufs=CFG["mm_bufs"],
                         name="ps_qk")
            for cb in range(NCB):
                nc.tensor.matmul(
                    ps, w_sb[:, cb, pr * P:(pr + 1) * P], x_sb[:, cb, :],
                    start=(cb == 0), stop=(cb == NCB - 1))
            nc.vector.tensor_scalar_add(
                xT_sb[:, pr, t0:t0 + TCH], ps, bias_sb[:, pr:pr + 1])
        return emit

    def v_unit(tb):
        def emit():
            ps = pp.tile([P, TCH], F32, tag="mm", bufs=CFG["mm_bufs"],
                         name="ps_v")
            for cb in range(NCB):
                nc.tensor.matmul(
                    ps[:, :GD], xv[:, cb, tb * P:(tb + 1) * P], wv_sb[:, cb, :],
                    start=(cb == 0), stop=(cb == NCB - 1))
            nc.vector.tensor_copy(
                v65_sb[:, t4 * (TCH // P) + tb, :, 0:DV],
                ps[:, :GD].rearrange("p (h d) -> p h d", h=HPG))
        return emit

    for pr in range(2):
        units.append(qk_unit(xq, wq_sb, bq_sb, qT_sb, pr))
    for pr in range(2):
        units.append(qk_unit(xk, wk_sb, bk_sb, kT_sb, pr))
    for tb in range(TCH // P):
        units.append(v_unit(tb))
    return units


def _wo_units(nc, io, c, qc, out_sb):
    """Output-projection work for one finished query chunk, as unit closures
    (interleaved into the NEXT chunk's PE stream)."""
    spool, pp = c["spool"], c["pp"]
    ident, wo_sb, outT_sb = c["ident"], c["wo"], c["outT"]
    units = []

    def trp_unit(pr):
        def emit():
            trp = pp.tile([P, 2 * TCH], BF, tag="mm", bufs=CFG["mm_bufs"],
                          name="trp")
            for qs in range(4):
                nc.tensor.transpose(
                    trp[:, qs * P:(qs + 1) * P],
                    out_sb[:, qs, 2 * pr:2 * pr + 2, :], ident)
            nc.vector.tensor_copy(
                outT_sb[:, pr, qc * TCH:(qc + 1) * TCH], trp[:, 0:TCH])
        return emit

    def fin_unit(tb):
        def emit():
            fin = spool.tile([P, C], OUT_DT, tag="fin", bufs=CFG["fin_bufs"],
                             name="fin")
            for cc in range(C // TCH):
                ps = pp.tile([P, TCH], F32, tag="mm", bufs=CFG["mm_bufs"],
                             name="ps_wo")
                for pr in range(2):
                    nc.tensor.matmul(
                        ps, outT_sb[:, pr, tb * P:(tb + 1) * P],
                        wo_sb[:, pr, cc * TCH:(cc + 1) * TCH],
                        start=(pr == 0), stop=(pr == 1))
                nc.vector.tensor_copy(fin[:, cc * TCH:(cc + 1) * TCH], ps)
            nc.sync.dma_start(out=io["out"][tb * P:(tb + 1) * P, :], in_=fin)
        return emit

    for pr in range(2):
        units.append(trp_unit(pr))
    for tb in range(qc * 4, qc * 4 + 4):
        units.append(fin_unit(tb))
    return units


def _emit_body(nc, tc, io, t_len, c, par, feeder, proj_par=None):
    """One full kernel body for rep-parity `par`.

    `feeder` is a list of pending unit closures (carried wo work); when
    `proj_par` is given, the projection units for THAT parity's next rep are
    appended chunk-by-chunk so they fill PE gaps during this rep's
    Act-bound attention phase.  Returns the leftover feeder (empty here —
    wo(3) is drained inline at body end).
    """
    NT = t_len // P
    NQC = t_len // TCH
    apool, pp = c["apool"], c["pp"]
    triT = c["triT"]
    qT_sb, kT_sb, v65_sb = c["qT"][par], c["kT"][par], c["v65"][par]

    def scores_pair(qc, h, kb0, expT):
        pr, hs = h // 2, (h % 2) * DK
        sc = pp.tile([P, 2 * TCH], F32, tag="sc", bufs=CFG["sc_bufs"],
                     name="sc")
        for j in range(2):
            kb = kb0 + j
            d = kb - qc * 4
            off = max(0, d) * P
            if CFG["fp8_scores"]:
                # DoubleRow: [32 part, 2, .] operands, 0.5 cycles/row.
                # tile_position passed explicitly: head 3 sits at partition
                # base 96, which the implicit base_partition() path rejects.
                for nh in range(2):
                    a = max(off, nh * 256)
                    b = (nh + 1) * 256
                    if a >= b:
                        continue
                    nc.tensor.matmul(
                        sc[:, j * TCH + a:j * TCH + b],
                        kT_sb[h * 32:(h + 1) * 32, :, kb * P:(kb + 1) * P],
                        qT_sb[h * 32:(h + 1) * 32, :,
                              qc * TCH + a:qc * TCH + b],
                        start=True, stop=True, perf_mode=DR,
                        tile_position=(h * 32, 0))
            else:
                nc.tensor.matmul(
                    sc[:, j * TCH + off:(j + 1) * TCH],
                    kT_sb[hs:hs + DK, pr, kb * P:(kb + 1) * P],
                    qT_sb[hs:hs + DK, pr, qc * TCH + off:(qc + 1) * TCH],
                    start=True, stop=True)
        if kb0 >= qc * 4:
            # diagonal pair: one exp per strip over its written range, then
            # zero the strict-lower (q < k) block part on idle GPSIMD
            for j in range(2):
                kb = kb0 + j
                d = kb - qc * 4
                nc.scalar.activation(
                    expT[:, kb * TCH + d * P:(kb + 1) * TCH],
                    sc[:, j * TCH + d * P:(j + 1) * TCH],
                    AF.Exp, scale=0.125)
            for j in range(2):
                kb = kb0 + j
                d = kb - qc * 4
                blk = expT[:, kb * TCH + d * P:kb * TCH + (d + 1) * P]
                nc.gpsimd.tensor_mul(blk, blk, triT)
        else:
            nc.scalar.activation(
                expT[:, kb0 * TCH:(kb0 + 2) * TCH], sc,
                AF.Exp, scale=0.125)

    def attnv_jobs(qc):
        # qs-major so each qs's PSUM accumulation group stays contiguous
        return [(qs, kb) for qs in range(4) for kb in range(qc * 4 + qs + 1)]

    def attnv_emit(qc, h, expT, out4, jobs):
        for qs, kb in jobs:
            qi = qc * 4 + qs
            nc.tensor.matmul(
                out4[:, qs, :],
                expT[:, kb * TCH + qs * P:kb * TCH + (qs + 1) * P],
                v65_sb[:, kb, h, :],
                start=(kb == 0), stop=(kb == qi))

    def attnv_norm(qc, h, out4, out_sb):
        R = apool.tile([P, 4], F32, tag="R", bufs=4, name="R")
        nc.vector.reciprocal(R, out4[:, :, DV])
        nc.vector.tensor_mul(
            out_sb[:, :, h, :], out4[:, :, 0:DV],
            R.unsqueeze(2).broadcast_to((P, 4, DV)))

    def attend_chunk(qc, chunk_units, start_slot=0):
        """Head-pipelined attention for one query chunk with `chunk_units`
        (proj/wo closures) interleaved between its score-pair strips,
        starting no earlier than `start_slot` (lets DMA-dependent units
        wait out their transfer without stalling the PE stream)."""
        out_sb = apool.tile([P, 4, HPG, DV], BF, tag="osb",
                            bufs=CFG["osb_bufs"], name="osb")
        pairs = list(range(0, (qc + 1) * 4, 2))
        npair = len(pairs)
        # slots: one per (head, pair) + npair for the trailing attnv
        nslots = (HPG + 1) * npair
        nu = len(chunk_units)
        span = max(1, nslots - start_slot)
        fed = 0

        def feed(slot):
            nonlocal fed
            want = max(0, slot + 1 - start_slot) * nu // span
            while fed < want:
                chunk_units[fed]()
                fed += 1

        slot = 0
        prev = None          # (h, expT, out4)
        for h in range(HPG):
            expT = apool.tile([P, NT * TCH], BF, tag="expT",
                              bufs=CFG["expt_bufs"], name="expT")
            jobs = attnv_jobs(qc) if prev is not None else []
            per = (len(jobs) + npair - 1) // npair if npair else 0
            for i, kb0 in enumerate(pairs):
                scores_pair(qc, h, kb0, expT)
                if prev is not None:
                    attnv_emit(qc, prev[0], prev[1], prev[2],
                               jobs[i * per:(i + 1) * per])
                feed(slot)
                slot += 1
            if prev is not None:
                attnv_norm(qc, prev[0], prev[2], out_sb)
            out4 = pp.tile([P, 4, DV + 1], F32, tag="out4",
                           bufs=CFG["out4_bufs"], name="out4")
            prev = (h, expT, out4)
        # trailing: last head's attnv in npair slices with feeding between
        jobs = attnv_jobs(qc)
        per = (len(jobs) + npair - 1) // npair
        for i in range(npair):
            attnv_emit(qc, prev[0], prev[1], prev[2],
                       jobs[i * per:(i + 1) * per])
            feed(slot)
            slot += 1
        while fed < nu:
            chunk_units[fed]()
            fed += 1
        attnv_norm(qc, prev[0], prev[2], out_sb)
        return out_sb

    if CFG["schedule"] == "block":
        # proven-on-HW schedule: project own parity as a block, then attend
        # chunks ascending with wo drained inline per chunk; cross-rep
        # overlap comes from parity buffers + staggered loop stages
        for t4 in range(NQC):
            for u in _proj_units(nc, io, c, par, t4, with_dma=True):
                u()
        for qc in range(NQC):
            out_sb = attend_chunk(qc, [])
            for u in _wo_units(nc, io, c, qc, out_sb):
                u()
        return feeder

    # Chunks are attended in DESCENDING order (3,2,1,0): the Act-heavy big
    # chunks come first and absorb the next rep's projection units as PE
    # filler; the wo units of each chunk are deferred into the next attended
    # chunk.  Input-DMA triggers fire at chunk heads, one-plus chunks ahead
    # of their consumers (xin ring WAR deps self-throttle the transfers).
    if proj_par is not None:
        streams = [_proj_units(nc, io, c, proj_par, t4, with_dma=True)
                   for t4 in range(NQC)]
        dmas = {3: [streams[3][0], streams[2][0]],
                2: [streams[1][0]], 1: [streams[0][0]], 0: []}
        projs = {3: streams[3][1:] + streams[2][1:],
                 2: streams[1][1:], 1: streams[0][1:], 0: []}
    else:
        dmas = {qc: [] for qc in range(NQC)}
        projs = {qc: [] for qc in range(NQC)}
    start_slots = {3: 10, 2: 0, 1: 0, 0: 0}

    for qc in reversed(range(NQC)):
        for dma_u in dmas[qc]:
            dma_u()
        chunk_units = list(feeder) + projs[qc]
        feeder.clear()
        out_sb = attend_chunk(qc, chunk_units, start_slots[qc])
        wo = _wo_units(nc, io, c, qc, out_sb)
        if qc > 0:
            feeder.extend(wo)
        else:
            for u in wo:       # last attended chunk: drain inline
                u()
    return feeder


def _build(t_len=T, reps=1, loop=True):
    nc = bacc.Bacc("TRN2", target_bir_lowering=False, debug=False,
                   num_devices=N_CORES)
    io = {
        "qT": nc.dram_tensor("qT", [C, t_len], BF, kind="ExternalInput"),
        "kT": nc.dram_tensor("kT", [C, t_len], BF, kind="ExternalInput"),
        "vT": nc.dram_tensor("vT", [C, t_len], BF, kind="ExternalInput"),
        "wq": nc.dram_tensor("wq", [C, GD], BF, kind="ExternalInput"),
        "wk": nc.dram_tensor("wk", [C, GD], BF, kind="ExternalInput"),
        "wv": nc.dram_tensor("wv", [C, GD], BF, kind="ExternalInput"),
        "wo": nc.dram_tensor("wo", [GD, C], BF, kind="ExternalInput"),
        "bq": nc.dram_tensor("bq", [P, 2], F32, kind="ExternalInput"),
        "bk": nc.dram_tensor("bk", [P, 2], F32, kind="ExternalInput"),
        "ident": nc.dram_tensor("ident", [P, P], BF, kind="ExternalInput"),
        "triT": nc.dram_tensor("triT", [P, P], BF, kind="ExternalInput"),
        "out": nc.dram_tensor("out", [t_len, C], OUT_DT, kind="ExternalOutput"),
    }
    hints = (mybir.EngineType.PE, mybir.EngineType.DVE,
             mybir.EngineType.Activation, mybir.EngineType.Pool,
             mybir.EngineType.SP)
    with tile.TileContext(nc) as tc, ExitStack() as ctx:
        c = _emit_consts(nc, tc, io, t_len, ctx)

        block = CFG["schedule"] == "block"

        def prologue(par):
            for t4 in range(t_len // TCH):
                for u in _proj_units(nc, io, c, par, t4, with_dma=True):
                    u()

        if not block:
            prologue(0)
        if reps == 1:
            _emit_body(nc, tc, io, t_len, c, 0, [])
        elif not loop:
            # straight-line unroll (steady-state simulation / analysis)
            for r in range(reps):
                par = r % 2
                nxt = (1 - par if r + 1 < reps else None) if not block else None
                _emit_body(nc, tc, io, t_len, c, par, [], proj_par=nxt)
        else:
            half, rem = divmod(reps, 2)
            pp1, pp0 = (None, None) if block else (1, 0)
            if half:
                with tc.For_i(0, half, 1, hint_engines=hints,
                              staggered_reset=CFG["staggered"]):
                    _emit_body(nc, tc, io, t_len, c, 0, [], proj_par=pp1)
                    _emit_body(nc, tc, io, t_len, c, 1, [], proj_par=pp0)
            for _ in range(rem):
                _emit_body(nc, tc, io, t_len, c, 0, [])
    nc.compile()
    return nc


_NC_CACHE = {}


def _get_nc(t_len=T, reps=1):
    key = (t_len, reps, tuple(sorted(CFG.items())))
    if key not in _NC_CACHE:
        _NC_CACHE[key] = _build(t_len, reps)
    return _NC_CACHE[key]


def _host_constants():
    ident = np.eye(P, dtype=bf16)
    # keep-mask: tri01[k, q] = 1 where q >= k (causal-valid), else 0
    triT = np.triu(np.ones((P, P), np.float32)).astype(bf16)
    return ident, triT


def make_in_maps(inputs, t_len=T):
    Q, K, V = inputs["Q"], inputs["K"], inputs["V"]
    Wq, bq = inputs["Wq"], inputs["bq"]
    Wk, bk = inputs["Wk"], inputs["bk"]
    Wv = inputs["Wv"]
    Wo = inputs["Wo"]
    ident, triT = _host_constants()
    qTs = [np.ascontiguousarray(Q[b, :t_len].T).astype(bf16) for b in range(B)]
    kTs = [np.ascontiguousarray(K[b, :t_len].T).astype(bf16) for b in range(B)]
    vTs = [np.ascontiguousarray(V[b, :t_len].T).astype(bf16) for b in range(B)]
    if CFG["fp8_scores"]:
        # column order for the DoubleRow q/k layout: matmul pr=i yields
        # partition rho = h*32+pp holding head-dim i*32+pp of head h
        perm = np.array([h * DK + i * 32 + p
                         for i in range(2) for h in range(HPG)
                         for p in range(32)])
    else:
        perm = np.arange(GD)
    in_maps = []
    for core in range(N_CORES):
        b, g = divmod(core, GROUPS)
        cs = slice(g * GD, (g + 1) * GD)
        in_maps.append({
            "qT": qTs[b],
            "kT": kTs[b],
            "vT": vTs[b],
            "wq": np.ascontiguousarray(Wq[:, cs][:, perm]).astype(bf16),
            "wk": np.ascontiguousarray(Wk[:, cs][:, perm]).astype(bf16),
            "wv": np.ascontiguousarray(Wv[:, cs]).astype(bf16),
            "wo": np.ascontiguousarray(Wo[cs, :]).astype(bf16),
            "bq": np.ascontiguousarray(
                bq[cs][perm].reshape(2, P).T).astype(np.float32),
            "bk": np.ascontiguousarray(
                bk[cs][perm].reshape(2, P).T).astype(np.float32),
            "ident": ident,
            "triT": triT,
        })
    return in_maps


def combine(results, inputs, t_len=T):
    bo, bv, Wo = inputs["bo"], inputs["bv"], inputs["Wo"]
    bias = (bo.astype(np.float64) + bv.astype(np.float64) @ Wo.astype(np.float64))
    out = np.empty((B, t_len, C), np.float32)
    for b in range(B):
        acc = np.zeros((t_len, C), np.float64)
        for g in range(GROUPS):
            acc += results[b * GROUPS + g]["out"].astype(np.float64)
        out[b] = (acc + bias).astype(np.float32)
    return out


def _mask_is_causal(mask, t_len):
    mask = np.asarray(mask)
    if mask.shape != (1, 1, t_len, t_len):
        return False
    m = mask[0, 0]
    tri = np.tril(np.ones((t_len, t_len), bool))
    return (m[tri] == 0.0).all() and (m[~tri] <= -1e8).all()


def _reference_fallback(inputs):
    # generic-mask fallback (never hit with the causal reference mask)
    Q, K, V = (np.asarray(inputs[k], np.float32) for k in ("Q", "K", "V"))
    mask = np.asarray(inputs["mask"], np.float32)
    out = np.empty((B, T, C), np.float32)
    for b in range(B):
        acc = np.zeros((T, C), np.float32)
        for h in range(H):
            q = Q[b] @ inputs["Wq"][:, h * DK:(h + 1) * DK] + inputs["bq"][h * DK:(h + 1) * DK]
            k = K[b] @ inputs["Wk"][:, h * DK:(h + 1) * DK] + inputs["bk"][h * DK:(h + 1) * DK]
            v = V[b] @ inputs["Wv"][:, h * DV:(h + 1) * DV] + inputs["bv"][h * DV:(h + 1) * DV]
            m = mask[min(b, mask.shape[0] - 1), min(h, mask.shape[1] - 1)]
            s = (q @ k.T + m) / np.sqrt(DK).astype(np.float32)
            s -= s.max(-1, keepdims=True)
            e = np.exp(s)
            a = e / e.sum(-1, keepdims=True)
            acc += (a @ v) @ inputs["Wo"][h * DV:(h + 1) * DV, :]
        out[b] = acc + inputs["bo"]
    return out


def kernel(**inputs):
    inputs = {k: np.asarray(v) for k, v in inputs.items()}
    if not _mask_is_causal(inputs["mask"], T):
        return _reference_fallback(inputs)
    nc = _get_nc(T)
    in_maps = make_in_maps(inputs, T)
    res = run_bass_kernel_spmd(nc, in_maps, core_ids=list(range(N_CORES)))
    return combine(res.results, inputs, T)


# revision 21
# speedup vs baseline: 1.3972x; 1.3972x over previous
"""Trainium2 Bass kernel: multi-head causal attention (B=2, T=2048, C=1024, H=16).

Sharding: 8 cores = data parallel over B (2) x tensor parallel over head
groups (4 groups of 4 heads).  Each core computes its batch's partial
output contribution from its 4 heads through Wo rows; the host sums the 4
partials per batch (the "all-reduce") and adds the folded biases.

Device pipeline (per core, 4 heads; matmul operands bf16, PSUM fp32):
  - Q/K/V arrive HOST-pre-transposed as [C, T] bf16, so projections need no
    on-chip transposes: qT/kT = Wq^T @ X^T laid out [head_dim, T] directly,
    v natural [T, dv] with an extra always-1.0 65th column per head.
  - scores are computed TRANSPOSED: scT[k, q] = kT_blk^T @ qT_chunk, one
    512-wide PE matmul per (key-block, query-chunk); diagonal strips are
    partial-width.
  - one Exp (scale=1/8) per PSUM pair-strip writes expT[k, q] bf16 (no
    normalization yet); the diagonal block's strict-lower (q < k) entries are
    then zeroed by a 0/1-triangle multiply on the idle GPSIMD engine.
  - weights/constants are DMA'd once outside the reps loop (SBUF-resident);
    heads are software-pipelined with attnv matmuls of head h-1 interleaved
    BETWEEN the score-pair strips of head h, so the in-order PE stream always
    has ready work while the Activation engine catches up on Exp.
  - attn@v: out[q, dv+1] accumulates expT_blk^T @ [v|1] over key blocks; the
    65th column is the softmax row-sum for free.  out = out[:, :64] * (1/sum)
    via one broadcast DVE multiply per (chunk, head).
  - per chunk: PE-transpose out -> outT[dims, q]; output projection
    fin[q, C] = outT^T @ Wo streams wide; DMA fin (bf16) to DRAM.
  - reps>1 (timing mode): the For_i body holds TWO kernel bodies with
    parity-alternated qT/kT/v65 tiles (breaks the cross-rep WAR hazard) and
    staggered_reset=True (no all-engine barrier at the back edge), so
    consecutive reps pipeline: rep i+1's input DMAs + projections overlap
    rep i's Act-bound attention tail.
"""

from contextlib import ExitStack

import numpy as np
import ml_dtypes

import concourse.bass as bass
import concourse.mybir as mybir
import concourse.tile as tile
from concourse import bacc
from concourse.bass_utils import run_bass_kernel_spmd

B, T, C = 2, 2048, 1024
H, DK, DV = 16, 64, 64
N_CORES = 8
GROUPS = 4                 # head groups (tensor parallel)
HPG = H // GROUPS          # 4 heads per group
GD = HPG * DK              # 256 head dims per group
P = 128
TCH = 512                  # query chunk for attention
NCB = C // P               # contraction chunks over C

BF = mybir.dt.bfloat16
F32 = mybir.dt.float32
AX = mybir.AxisListType
AF = mybir.ActivationFunctionType

bf16 = ml_dtypes.bfloat16

CFG = {"xin_bufs": 2, "sc_bufs": 3, "mm_bufs": 1, "out4_bufs": 1,
       "expt_bufs": 3, "fin_bufs": 3, "osb_bufs": 2,
       "parity": True, "staggered": True, "out_bf16": True,
       # "block": proj block at body start (cross-rep overlap via parity +
       # staggered stages); "desc": descending chunks with fine-grained
       # proj/wo unit interleave (better in the cost model, worse on HW)
       "schedule": "block",
       # fp8e4m3 q/k + DoubleRow scores matmuls (0.5 cycles/row on PE).
       # Measured SLOWER on HW (150.3us vs 128.8): the 256-wide DoubleRow
       # stationary loads exceed their 128-cycle matmuls (Ldweights-bound).
       "fp8_scores": False}

OUT_DT = BF if CFG["out_bf16"] else F32
F8 = mybir.dt.float8e4
QK_DT = F8 if CFG["fp8_scores"] else BF
DR = mybir.MatmulPerfMode.DoubleRow


def _emit_consts(nc, tc, io, t_len, ctx):
    """Loop-invariant setup: weights/constants DMA + persistent activation
    tiles (x2 for rep-parity double buffering) + the ones column."""
    NT = t_len // P
    cpool = ctx.enter_context(tc.tile_pool(name="const", bufs=1))
    ppool = ctx.enter_context(tc.tile_pool(name="pers", bufs=1))

    ident = cpool.tile([P, P], BF)
    nc.sync.dma_start(out=ident, in_=io["ident"][:, :])
    triT = cpool.tile([P, P], BF)   # keep-mask 0/1 on the causal diagonal
    nc.sync.dma_start(out=triT, in_=io["triT"][:, :])
    bq_sb = cpool.tile([P, 2], F32)
    nc.sync.dma_start(out=bq_sb, in_=io["bq"][:, :])
    bk_sb = cpool.tile([P, 2], F32)
    nc.sync.dma_start(out=bk_sb, in_=io["bk"][:, :])

    wq_sb = cpool.tile([P, NCB, GD], BF)
    wk_sb = cpool.tile([P, NCB, GD], BF)
    wv_sb = cpool.tile([P, NCB, GD], BF)
    for w_sb, name in ((wq_sb, "wq"), (wk_sb, "wk"), (wv_sb, "wv")):
        nc.sync.dma_start(
            out=w_sb,
            in_=io[name][:, :].rearrange("(c p) d -> p c d", p=P))
    wo_sb = cpool.tile([P, 2, C], BF)
    nc.sync.dma_start(
        out=wo_sb, in_=io["wo"][:, :].rearrange("(r p) d -> p r d", p=P))

    npar = 2 if CFG["parity"] else 1
    # fp8 mode: partition rho = h*32+pp, free (i, t) holds head-dim i*32+pp
    # (host permutes Wq/Wk columns to match) — the DoubleRow scores layout
    qT_sb = [ppool.tile([P, 2, t_len], QK_DT, name=f"qT_sb{i}")
             for i in range(npar)]
    kT_sb = [ppool.tile([P, 2, t_len], QK_DT, name=f"kT_sb{i}")
             for i in range(npar)]
    v65_sb = [ppool.tile([P, NT, HPG, DV + 1], BF, name=f"v65_sb{i}")
              for i in range(npar)]
    outT_sb = ppool.tile([P, 2, t_len], BF)  # [pair head dims, pair, T]
    for v65 in v65_sb:
        nc.gpsimd.memset(v65[:, :, :, DV:DV + 1], 1.0)

    # shared streaming pools (created ONCE; both parity bodies rotate the
    # same tag ring buffers, which also gives cross-body WAR tracking)
    spool = ctx.enter_context(tc.tile_pool(name="stream", bufs=2))
    apool = ctx.enter_context(tc.tile_pool(name="attn", bufs=2))
    pp = ctx.enter_context(tc.tile_pool(name="ps", bufs=2, space="PSUM"))

    return {"ident": ident, "triT": triT, "bq": bq_sb, "bk": bk_sb,
            "wq": wq_sb, "wk": wk_sb, "wv": wv_sb, "wo": wo_sb,
            "qT": qT_sb, "kT": kT_sb, "v65": v65_sb, "outT": outT_sb,
            "spool": spool, "apool": apool, "pp": pp}


def _proj_units(nc, io, c, par, t4, with_dma):
    """One query-chunk's worth of projection work as a list of closures.

    Each unit is an atomic emission (one PSUM accumulation group + its
    drain); the caller interleaves units into the attention PE stream so the
    in-order PE queue always has ready work while Activation catches up on
    Exp and DVE drains the previous PSUM group.
    """
    spool, pp = c["spool"], c["pp"]
    bq_sb, bk_sb = c["bq"], c["bk"]
    wq_sb, wk_sb, wv_sb = c["wq"], c["wk"], c["wv"]
    qT_sb, kT_sb, v65_sb = c["qT"][par], c["kT"][par], c["v65"][par]
    t0 = t4 * TCH
    xq = spool.tile([P, NCB, TCH], BF, tag="xq", bufs=CFG["xin_bufs"],
                    name=f"xq{par}_{t4}")
    xk = spool.tile([P, NCB, TCH], BF, tag="xk", bufs=CFG["xin_bufs"],
                    name=f"xk{par}_{t4}")
    xv = spool.tile([P, NCB, TCH], BF, tag="xv", bufs=CFG["xin_bufs"],
                    name=f"xv{par}_{t4}")
    units = []

    def dma_unit():
        for x_sb, name in ((xq, "qT"), (xk, "kT"), (xv, "vT")):
            nc.sync.dma_start(
                out=x_sb,
                in_=io[name][:, t0:t0 + TCH].rearrange(
                    "(c p) t -> p c t", p=P))
    if with_dma:
        units.append(dma_unit)

    def qk_unit(x_sb, w_sb, bias_sb, xT_sb, pr):
        def emit():
            ps = pp.tile([P, TCH], F32, tag="mm", bufs=CFG["mm_bufs"],
                         name="ps_qk")
            for cb in range(NCB):
                nc.tensor.matmul(
                    ps, w_sb[:, cb, pr * P:(pr + 1) * P], x_sb[:, cb, :],
                    start=(cb == 0), stop=(cb == NCB - 1))
            nc.vector.tensor_scalar_add(
                xT_sb[:, pr, t0:t0 + TCH], ps, bias_sb[:, pr:pr + 1])
        return emit

    def v_unit(tb):
        def emit():
            ps = pp.tile([P, TCH], F32, tag="mm", bufs=CFG["mm_bufs"],
                         name="ps_v")
            for cb in range(NCB):
                nc.tensor.matmul(
                    ps[:, :GD], xv[:, cb, tb * P:(tb + 1) * P], wv_sb[:, cb, :],
                    start=(cb == 0), stop=(cb == NCB - 1))
            nc.vector.tensor_copy(
                v65_sb[:, t4 * (TCH // P) + tb, :, 0:DV],
                ps[:, :GD].rearrange("p (h d) -> p h d", h=HPG))
        return emit

    for pr in range(2):
        units.append(qk_unit(xq, wq_sb, bq_sb, qT_sb, pr))
    for pr in range(2):
        units.append(qk_unit(xk, wk_sb, bk_sb, kT_sb, pr))
    for tb in range(TCH // P):
        units.append(v_unit(tb))
    return units


def _wo_units(nc, io, c, qc, out_sb):
    """Output-projection work for one finished query chunk, as unit closures
    (interleaved into the NEXT chunk's PE stream)."""
    spool, pp = c["spool"], c["pp"]
    ident, wo_sb, outT_sb = c["ident"], c["wo"], c["outT"]
    units = []

    def trp_unit(pr):
        def emit():
            trp = pp.tile([P, 2 * TCH], BF, tag="mm", bufs=CFG["mm_bufs"],
                          name="trp")
            for qs in range(4):
                nc.tensor.transpose(
                    trp[:, qs * P:(qs + 1) * P],
                    out_sb[:, qs, 2 * pr:2 * pr + 2, :], ident)
            nc.vector.tensor_copy(
                outT_sb[:, pr, qc * TCH:(qc + 1) * TCH], trp[:, 0:TCH])
        return emit

    def fin_unit(tb):
        def emit():
            fin = spool.tile([P, C], OUT_DT, tag="fin", bufs=CFG["fin_bufs"],
                             name="fin")
            for cc in range(C // TCH):
                ps = pp.tile([P, TCH], F32, tag="mm", bufs=CFG["mm_bufs"],
                             name="ps_wo")
                for pr in range(2):
                    nc.tensor.matmul(
                        ps, outT_sb[:, pr, tb * P:(tb + 1) * P],
                        wo_sb[:, pr, cc * TCH:(cc + 1) * TCH],
                        start=(pr == 0), stop=(pr == 1))
                nc.vector.tensor_copy(fin[:, cc * TCH:(cc + 1) * TCH], ps)
            nc.sync.dma_start(out=io["out"][tb * P:(tb + 1) * P, :], in_=fin)
        return emit

    for pr in range(2):
        units.append(trp_unit(pr))
    for tb in range(qc * 4, qc * 4 + 4):
        units.append(fin_unit(tb))
    return units


def _emit_body(nc, tc, io, t_len, c, par, feeder, proj_par=None):
    """One full kernel body for rep-parity `par`.

    `feeder` is a list of pending unit closures (carried wo work); when
    `proj_par` is given, the projection units for THAT parity's next rep are
    appended chunk-by-chunk so they fill PE gaps during this rep's
    Act-bound attention phase.  Returns the leftover feeder (empty here —
    wo(3) is drained inline at body end).
    """
    NT = t_len // P
    NQC = t_len // TCH
    apool, pp = c["apool"], c["pp"]
    triT = c["triT"]
    qT_sb, kT_sb, v65_sb = c["qT"][par], c["kT"][par], c["v65"][par]

    def scores_pair(qc, h, kb0, expT):
        pr, hs = h // 2, (h % 2) * DK
        sc = pp.tile([P, 2 * TCH], F32, tag="sc", bufs=CFG["sc_bufs"],
                     name="sc")
        for j in range(2):
            kb = kb0 + j
            d = kb - qc * 4
            off = max(0, d) * P
            if CFG["fp8_scores"]:
                # DoubleRow: [32 part, 2, .] operands, 0.5 cycles/row.
                # tile_position passed explicitly: head 3 sits at partition
                # base 96, which the implicit base_partition() path rejects.
                for nh in range(2):
                    a = max(off, nh * 256)
                    b = (nh + 1) * 256
                    if a >= b:
                        continue
                    nc.tensor.matmul(
                        sc[:, j * TCH + a:j * TCH + b],
                        kT_sb[h * 32:(h + 1) * 32, :, kb * P:(kb + 1) * P],
                        qT_sb[h * 32:(h + 1) * 32, :,
                              qc * TCH + a:qc * TCH + b],
                        start=True, stop=True, perf_mode=DR,
                        tile_position=(h * 32, 0))
            else:
                nc.tensor.matmul(
                    sc[:, j * TCH + off:(j + 1) * TCH],
                    kT_sb[hs:hs + DK, pr, kb * P:(kb + 1) * P],
                    qT_sb[hs:hs + DK, pr, qc * TCH + off:(qc + 1) * TCH],
                    start=True, stop=True)
        if kb0 >= qc * 4:
            # diagonal pair: one exp per strip over its written range, then
            # zero the strict-lower (q < k) block part on idle GPSIMD
            for j in range(2):
                kb = kb0 + j
                d = kb - qc * 4
                nc.scalar.activation(
                    expT[:, kb * TCH + d * P:(kb + 1) * TCH],
                    sc[:, j * TCH + d * P:(j + 1) * TCH],
                    AF.Exp, scale=0.125)
            for j in range(2):
                kb = kb0 + j
                d = kb - qc * 4
                blk = expT[:, kb * TCH + d * P:kb * TCH + (d + 1) * P]
                nc.gpsimd.tensor_mul(blk, blk, triT)
        else:
            nc.scalar.activation(
                expT[:, kb0 * TCH:(kb0 + 2) * TCH], sc,
                AF.Exp, scale=0.125)

    def attnv_jobs(qc):
        # qs-major so each qs's PSUM accumulation group stays contiguous
        return [(qs, kb) for qs in range(4) for kb in range(qc * 4 + qs + 1)]

    def attnv_emit(qc, h, expT, out4, jobs):
        for qs, kb in jobs:
            qi = qc * 4 + qs
            nc.tensor.matmul(
                out4[:, qs, :],
                expT[:, kb * TCH + qs * P:kb * TCH + (qs + 1) * P],
                v65_sb[:, kb, h, :],
                start=(kb == 0), stop=(kb == qi))

    def attnv_norm(qc, h, out4, out_sb):
        R = apool.tile([P, 4], F32, tag="R", bufs=4, name="R")
        nc.vector.reciprocal(R, out4[:, :, DV])
        nc.vector.tensor_mul(
            out_sb[:, :, h, :], out4[:, :, 0:DV],
            R.unsqueeze(2).broadcast_to((P, 4, DV)))

    def attend_chunk(qc, chunk_units, start_slot=0):
        """Head-pipelined attention for one query chunk with `chunk_units`
        (proj/wo closures) interleaved between its score-pair strips,
        starting no earlier than `start_slot` (lets DMA-dependent units
        wait out their transfer without stalling the PE stream)."""
        out_sb = apool.tile([P, 4, HPG, DV], BF, tag="osb",
                            bufs=CFG["osb_bufs"], name="osb")
        pairs = list(range(0, (qc + 1) * 4, 2))
        npair = len(pairs)
        # slots: one per (head, pair) + npair for the trailing attnv
        nslots = (HPG + 1) * npair
        nu = len(chunk_units)
        span = max(1, nslots - start_slot)
        fed = 0

        def feed(slot):
            nonlocal fed
            want = max(0, slot + 1 - start_slot) * nu // span
            while fed < want:
                chunk_units[fed]()
                fed += 1

        slot = 0
        prev = None          # (h, expT, out4)
        for h in range(HPG):
            expT = apool.tile([P, NT * TCH], BF, tag="expT",
                              bufs=CFG["expt_bufs"], name="expT")
            jobs = attnv_jobs(qc) if prev is not None else []
            per = (len(jobs) + npair - 1) // npair if npair else 0
            for i, kb0 in enumerate(pairs):
                scores_pair(qc, h, kb0, expT)
                if prev is not None:
                    attnv_emit(qc, prev[0], prev[1], prev[2],
                               jobs[i * per:(i + 1) * per])
                feed(slot)
                slot += 1
            if prev is not None:
                attnv_norm(qc, prev[0], prev[2], out_sb)
            out4 = pp.tile([P, 4, DV + 1], F32, tag="out4",
                           bufs=CFG["out4_bufs"], name="out4")
            prev = (h, expT, out4)
        # trailing: last head's attnv in npair slices with feeding between
        jobs = attnv_jobs(qc)
        per = (len(jobs) + npair - 1) // npair
        for i in range(npair):
            attnv_emit(qc, prev[0], prev[1], prev[2],
                       jobs[i * per:(i + 1) * per])
            feed(slot)
            slot += 1
        while fed < nu:
            chunk_units[fed]()
            fed += 1
        attnv_norm(qc, prev[0], prev[2], out_sb)
        return out_sb

    if CFG["schedule"] == "block":
        # proven-on-HW schedule: project own parity as a block, then attend
        # chunks ascending with wo drained inline per chunk; cross-rep
        # overlap comes from parity buffers + staggered loop stages
        for t4 in range(NQC):
            for u in _proj_units(nc, io, c, par, t4, with_dma=True):
                u()
        for qc in range(NQC):
            out_sb = attend_chunk(qc, [])
            for u in _wo_units(nc, io, c, qc, out_sb):
                u()
        return feeder

    # Chunks are attended in DESCENDING order (3,2,1,0): the Act-heavy big
    # chunks come first and absorb the next rep's projection units as PE
    # filler; the wo units of each chunk are deferred into the next attended
    # chunk.  Input-DMA triggers fire at chunk heads, one-plus chunks ahead
    # of their consumers (xin ring WAR deps self-throttle the transfers).
    if proj_par is not None:
        streams = [_proj_units(nc, io, c, proj_par, t4, with_dma=True)
                   for t4 in range(NQC)]
        dmas = {3: [streams[3][0], streams[2][0]],
                2: [streams[1][0]], 1: [streams[0][0]], 0: []}
        projs = {3: streams[3][1:] + streams[2][1:],
                 2: streams[1][1:], 1: streams[0][1:], 0: []}
    else:
        dmas = {qc: [] for qc in range(NQC)}
        projs = {qc: [] for qc in range(NQC)}
    start_slots = {3: 10, 2: 0, 1: 0, 0: 0}

    for qc in reversed(range(NQC)):
        for dma_u in dmas[qc]:
            dma_u()
        chunk_units = list(feeder) + projs[qc]
        feeder.clear()
        out_sb = attend_chunk(qc, chunk_units, start_slots[qc])
        wo = _wo_units(nc, io, c, qc, out_sb)
        if qc > 0:
            feeder.extend(wo)
        else:
            for u in wo:       # last attended chunk: drain inline
                u()
    return feeder


def _build(t_len=T, reps=1, loop=True):
    nc = bacc.Bacc("TRN2", target_bir_lowering=False, debug=False,
                   num_devices=N_CORES)
    io = {
        "qT": nc.dram_tensor("qT", [C, t_len], BF, kind="ExternalInput"),
        "kT": nc.dram_tensor("kT", [C, t_len], BF, kind="ExternalInput"),
        "vT": nc.dram_tensor("vT", [C, t_len], BF, kind="ExternalInput"),
        "wq": nc.dram_tensor("wq", [C, GD], BF, kind="ExternalInput"),
        "wk": nc.dram_tensor("wk", [C, GD], BF, kind="ExternalInput"),
        "wv": nc.dram_tensor("wv", [C, GD], BF, kind="ExternalInput"),
        "wo": nc.dram_tensor("wo", [GD, C], BF, kind="ExternalInput"),
        "bq": nc.dram_tensor("bq", [P, 2], F32, kind="ExternalInput"),
        "bk": nc.dram_tensor("bk", [P, 2], F32, kind="ExternalInput"),
        "ident": nc.dram_tensor("ident", [P, P], BF, kind="ExternalInput"),
        "triT": nc.dram_tensor("triT", [P, P], BF, kind="ExternalInput"),
        "out": nc.dram_tensor("out", [t_len, C], OUT_DT, kind="ExternalOutput"),
    }
    hints = (mybir.EngineType.PE, mybir.EngineType.DVE,
             mybir.EngineType.Activation, mybir.EngineType.Pool,
             mybir.EngineType.SP)
    with tile.TileContext(nc) as tc, ExitStack() as ctx:
        c = _emit_consts(nc, tc, io, t_len, ctx)

        block = CFG["schedule"] == "block"

        def prologue(par):
            for t4 in range(t_len // TCH):
                for u in _proj_units(nc, io, c, par, t4, with_dma=True):
                    u()

        if not block:
            prologue(0)
        if reps == 1:
            _emit_body(nc, tc, io, t_len, c, 0, [])
        elif not loop:
            # straight-line unroll (steady-state simulation / analysis)
            for r in range(reps):
                par = r % 2
                nxt = (1 - par if r + 1 < reps else None) if not block else None
                _emit_body(nc, tc, io, t_len, c, par, [], proj_par=nxt)
        else:
            half, rem = divmod(reps, 2)
            pp1, pp0 = (None, None) if block else (1, 0)
            if half:
                with tc.For_i(0, half, 1, hint_engines=hints,
                              staggered_reset=CFG["staggered"]):
                    _emit_body(nc, tc, io, t_len, c, 0, [], proj_par=pp1)
                    _emit_body(nc, tc, io, t_len, c, 1, [], proj_par=pp0)
            for _ in range(rem):
                _emit_body(nc, tc, io, t_len, c, 0, [])
    nc.compile()
    return nc


_NC_CACHE = {}


def _get_nc(t_len=T, reps=1):
    key = (t_len, reps, tuple(sorted(CFG.items())))
    if key not in _NC_CACHE:
        _NC_CACHE[key] = _build(t_len, reps)
    return _NC_CACHE[key]


def _host_constants():
    ident = np.eye(P, dtype=bf16)
    # keep-mask: tri01[k, q] = 1 where q >= k (causal-valid), else 0
    triT = np.triu(np.ones((P, P), np.float32)).astype(bf16)
    return ident, triT


def make_in_maps(inputs, t_len=T):
    Q, K, V = inputs["Q"], inputs["K"], inputs["V"]
    Wq, bq = inputs["Wq"], inputs["bq"]
    Wk, bk = inputs["Wk"], inputs["bk"]
    Wv = inputs["Wv"]
    Wo = inputs["Wo"]
    ident, triT = _host_constants()
    qTs = [np.ascontiguousarray(Q[b, :t_len].T).astype(bf16) for b in range(B)]
    kTs = [np.ascontiguousarray(K[b, :t_len].T).astype(bf16) for b in range(B)]
    vTs = [np.ascontiguousarray(V[b, :t_len].T).astype(bf16) for b in range(B)]
    if CFG["fp8_scores"]:
        # column order for the DoubleRow q/k layout: matmul pr=i yields
        # partition rho = h*32+pp holding head-dim i*32+pp of head h
        perm = np.array([h * DK + i * 32 + p
                         for i in range(2) for h in range(HPG)
                         for p in range(32)])
    else:
        perm = np.arange(GD)
    in_maps = []
    for core in range(N_CORES):
        b, g = divmod(core, GROUPS)
        cs = slice(g * GD, (g + 1) * GD)
        in_maps.append({
            "qT": qTs[b],
            "kT": kTs[b],
            "vT": vTs[b],
            "wq": np.ascontiguousarray(Wq[:, cs][:, perm]).astype(bf16),
            "wk": np.ascontiguousarray(Wk[:, cs][:, perm]).astype(bf16),
            "wv": np.ascontiguousarray(Wv[:, cs]).astype(bf16),
            "wo": np.ascontiguousarray(Wo[cs, :]).astype(bf16),
            "bq": np.ascontiguousarray(
                bq[cs][perm].reshape(2, P).T).astype(np.float32),
            "bk": np.ascontiguousarray(
                bk[cs][perm].reshape(2, P).T).astype(np.float32),
            "ident": ident,
            "triT": triT,
        })
    return in_maps


def combine(results, inputs, t_len=T):
    bo, bv, Wo = inputs["bo"], inputs["bv"], inputs["Wo"]
    bias = (bo.astype(np.float64) + bv.astype(np.float64) @ Wo.astype(np.float64))
    out = np.empty((B, t_len, C), np.float32)
    for b in range(B):
        acc = np.zeros((t_len, C), np.float64)
        for g in range(GROUPS):
            acc += results[b * GROUPS + g]["out"].astype(np.float64)
        out[b] = (acc + bias).astype(np.float32)
    return out


def _mask_is_causal(mask, t_len):
    mask = np.asarray(mask)
    if mask.shape != (1, 1, t_len, t_len):
        return False
    m = mask[0, 0]
    tri = np.tril(np.ones((t_len, t_len), bool))
    return (m[tri] == 0.0).all() and (m[~tri] <= -1e8).all()


def _reference_fallback(inputs):
    # generic-mask fallback (never hit with the causal reference mask)
    Q, K, V = (np.asarray(inputs[k], np.float32) for k in ("Q", "K", "V"))
    mask = np.asarray(inputs["mask"], np.float32)
    out = np.empty((B, T, C), np.float32)
    for b in range(B):
        acc = np.zeros((T, C), np.float32)
        for h in range(H):
            q = Q[b] @ inputs["Wq"][:, h * DK:(h + 1) * DK] + inputs["bq"][h * DK:(h + 1) * DK]
            k = K[b] @ inputs["Wk"][:, h * DK:(h + 1) * DK] + inputs["bk"][h * DK:(h + 1) * DK]
            v = V[b] @ inputs["Wv"][:, h * DV:(h + 1) * DV] + inputs["bv"][h * DV:(h + 1) * DV]
            m = mask[min(b, mask.shape[0] - 1), min(h, mask.shape[1] - 1)]
            s = (q @ k.T + m) / np.sqrt(DK).astype(np.float32)
            s -= s.max(-1, keepdims=True)
            e = np.exp(s)
            a = e / e.sum(-1, keepdims=True)
            acc += (a @ v) @ inputs["Wo"][h * DV:(h + 1) * DV, :]
        out[b] = acc + inputs["bo"]
    return out


def kernel(**inputs):
    inputs = {k: np.asarray(v) for k, v in inputs.items()}
    if not _mask_is_causal(inputs["mask"], T):
        return _reference_fallback(inputs)
    nc = _get_nc(T)
    in_maps = make_in_maps(inputs, T)
    res = run_bass_kernel_spmd(nc, in_maps, core_ids=list(range(N_CORES)))
    return combine(res.results, inputs, T)
